# revision 51
# baseline (speedup 1.0000x reference)
"""Trainium2 Bass kernel: causal self-attention with RoPE (16 heads, B=2, S=2048, D=2048).

Sharding: 8 cores = 2 (batch, data-parallel) x 4 (head-groups of 4 heads, tensor
parallel).  Each core computes q/k/v projections for its 4 heads, RoPE, causal
attention, and a partial o_proj over its 512 rows of Wo.  The 4 partial [S, D]
outputs per batch are summed on the host (the "all-reduce" of o_proj).

All matmuls run in bf16 (cast on host), psum accumulation in f32.  Structure:
  - single x window resident in SBUF (bf16), weights streamed once
  - softmax denominator fused into the PV matmul: pt (exp scores) is the
    stationary operand, moving operand is [v | 1] (ones column baked into the
    v SBUF layout at stride 129), so the row-sum lands in psum col 128 free
  - causal diag blocks compute only the valid suffix; triangular boundary
    chunks masked post-exp on DVE
  - attention output [i, d] flipped to [d, i] via DMA-XBAR transpose
  - head-major software pipeline: attn(h) interleaved with qkv(h+1) on the
    PE queue so the tensor engine never waits on Act exp; st runs two key
    blocks ahead of pv
"""

import math

import numpy as np

# ---- problem constants ----
B, S, D = 2, 2048, 2048
NUM_HEADS, HD = 16, 128
N_CORES = 8
GROUPS = 4                  # head-groups (tensor-parallel)
H_PER_CORE = NUM_HEADS // GROUPS   # 4
E_PER_CORE = H_PER_CORE * HD       # 512

_CACHE = {}


# --------------------------------------------------------------------------
# host-side helpers
# --------------------------------------------------------------------------

def _rope_sin_cos(seq_len, head_dim):
    """float32, matches reference._rope_sin_cos."""
    pos = np.arange(seq_len, dtype=np.float32)
    freq_seq = np.arange(0, head_dim, 2, dtype=np.float32)
    inv_freq = (np.float32(1.0) / (np.float32(10000.0) ** (freq_seq / np.float32(head_dim)))).astype(np.float32)
    sinusoid = pos[:, None] * inv_freq[None, :]          # [S, hd/2]
    return np.sin(sinusoid).astype(np.float32), np.cos(sinusoid).astype(np.float32)


def _rope_tables(seq_len):
    """CC / SS' [128, seq_len] f32 in the quadrant-paired layout.
    CC row = cos(pair angle) at both x1 and x2 rows.
    SS' = +sin at x1 rows, -sin at x2 rows, so that
    shuffle16(ps*SS') = [-x2*sin at x1 rows ; x1*sin at x2 rows]."""
    sin, cos = _rope_sin_cos(seq_len, HD)       # [S, 64]
    cosT = cos.T                                # [64, S] pair-index order
    sinT = sin.T
    x1, x2 = _pair_pos()
    CC = np.empty((HD, seq_len), dtype=np.float32)
    SS = np.empty((HD, seq_len), dtype=np.float32)
    CC[x1] = cosT
    CC[x2] = cosT
    SS[x1] = sinT
    SS[x2] = -sinT
    return CC, SS


def _deinterleave_idx():
    """Row permutation within one head: quadrant-paired so the RoPE partner
    swap is a within-quadrant rotation by 16 (DVE stream_shuffle-able)."""
    idx = np.empty(HD, dtype=np.int64)
    for q in range(4):
        t = 16 * q + np.arange(16)
        idx[q * 32:q * 32 + 16] = 2 * t
        idx[q * 32 + 16:q * 32 + 32] = 2 * t + 1
    return idx


def _pair_pos():
    """(x1_rows, x2_rows) in the deinterleaved layout, pair-index order."""
    x1 = np.concatenate([q * 32 + np.arange(16) for q in range(4)])
    x2 = x1 + 16
    return x1, x2


def _np_rope_apply(q, sin, cos):
    """q: [S, 128] in the quadrant-paired deinterleaved layout."""
    p1, p2 = _pair_pos()
    x1, x2 = q[:, p1], q[:, p2]
    r = np.empty_like(q)
    r[:, p1] = x1 * cos - x2 * sin
    r[:, p2] = x1 * sin + x2 * cos
    return r


def _np_core_model(xT, wq, wk, wv, wo):
    """Numpy model of what ONE core's device program computes (f32 version
    of the math; device uses bf16)."""
    Dm, S_ = xT.shape
    E_ = wq.shape[1]
    H_ = E_ // HD
    x = xT.T.astype(np.float32)
    sin, cos = _rope_sin_cos(S_, HD)
    out = np.zeros((S_, Dm), dtype=np.float32)
    causal = np.tril(np.ones((S_, S_), dtype=bool))
    for h in range(H_):
        q = x @ wq[:, h * HD:(h + 1) * HD]
        k = x @ wk[:, h * HD:(h + 1) * HD]
        v = x @ wv[:, h * HD:(h + 1) * HD]
        q = _np_rope_apply(q, sin, cos)
        k = _np_rope_apply(k, sin, cos)
        s = (q @ k.T) / math.sqrt(HD)
        s = np.where(causal, s, -np.inf)
        p = np.exp(s - s.max(axis=-1, keepdims=True))
        p = p / p.sum(axis=-1, keepdims=True)
        out += (p @ v) @ wo[h * HD:(h + 1) * HD, :]
    return out


def _np_reference(x, Wq, Wk, Wv, Wo, attn_mask):
    """Full-problem numpy fallback replicating reference.py (generic mask)."""
    B_, S_, D_ = x.shape
    H = NUM_HEADS
    hd = D_ // H
    sin, cos = _rope_sin_cos(S_, hd)

    def proj(W):
        y = np.einsum('bsd,ed->bse', x, W)
        return y.reshape(B_, S_, H, hd).transpose(0, 2, 1, 3)

    q, k, v = proj(Wq), proj(Wk), proj(Wv)

    def rope(t):
        tr = t.reshape(B_, H, S_, hd // 2, 2)
        x1, x2 = tr[..., 0], tr[..., 1]
        r1 = x1 * cos[None, None] - x2 * sin[None, None]
        r2 = x1 * sin[None, None] + x2 * cos[None, None]
        return np.stack((r1, r2), axis=-1).reshape(B_, H, S_, hd)

    q, k = rope(q), rope(k)
    scores = np.einsum('bhqd,bhkd->bhqk', q, k) / math.sqrt(hd) + attn_mask
    scores = scores - scores.max(axis=-1, keepdims=True)
    p = np.exp(scores)
    p = p / p.sum(axis=-1, keepdims=True)
    attn = np.einsum('bhqk,bhkd->bhqd', p, v)
    attn = attn.transpose(0, 2, 1, 3).reshape(B_, S_, D_)
    return np.einsum('bsd,ed->bse', attn, Wo)


# --------------------------------------------------------------------------
# device program builder
# --------------------------------------------------------------------------

def build(S_=S, Dm_=D, H_=H_PER_CORE, mmdt="bf16"):
    """Build the per-core Bass program (SPMD: same program, 8 data shards).

    Inputs (DRAM, bf16, host pre-tiled to [128, ...] partition-major):
      xP   [128, DT*S_]        x tiles, d-major (xP[p, d*S_+s] = x[s, d*128+p])
      wqkP [128, 2*H_*DT*128]  per (h,q|k) block of DT d-tiles of [128,128]
      wvP  [128, H_*DT*128]    per h block of DT d-tiles
      woP  [128, H_*Dm_]       woP[p, h*Dm_+j] = Wo_core[h*128+p, j]
    Output (DRAM): out [S_, Dm_] bf16 (partial o_proj; host sums groups)
    """
    import concourse.tile as tile
    from concourse import bacc, mybir
    import ml_dtypes

    f32 = mybir.dt.float32
    bf16 = mybir.dt.bfloat16
    AF = mybir.ActivationFunctionType

    DT = Dm_ // 128            # contraction tiles
    NG = S_ // 512             # attention i-groups
    SBK = S_ // 128            # s-blocks
    NDB = Dm_ // 512           # o_proj D chunks
    NSW = S_ // 512            # qk projection s-swaths
    SCALE = 1.0 / math.sqrt(HD)
    SWAP16 = [(i + 16) % 32 for i in range(32)]
    VW = H_ * 129              # v sbuf tile width (ones col per head)

    nc = bacc.Bacc("TRN2", target_bir_lowering=False, debug=False)

    xP_d = nc.dram_tensor("xP", [128, DT * S_], bf16, kind="ExternalInput")
    wqkP_d = nc.dram_tensor("wqkP", [128, 2 * H_ * DT * 128], bf16, kind="ExternalInput")
    wvP_d = nc.dram_tensor("wvP", [128, H_ * DT * 128], bf16, kind="ExternalInput")
    woP_d = nc.dram_tensor("woP", [128, H_ * Dm_], bf16, kind="ExternalInput")
    out_d = nc.dram_tensor("out", [S_, Dm_], bf16, kind="ExternalOutput")

    CC_np, SS_np = _rope_tables(S_)
    cc_dram = nc.inline_tensor(CC_np.astype(ml_dtypes.bfloat16), "cc_const")
    ss_dram = nc.inline_tensor(SS_np.astype(ml_dtypes.bfloat16), "ss_const")
    tri_np = np.triu(np.ones((128, 128), dtype=np.float32)).astype(ml_dtypes.bfloat16)
    tri_dram = nc.inline_tensor(tri_np, "tri_const")

    from contextlib import ExitStack

    with tile.TileContext(nc) as tc, ExitStack() as ctx:
        # ---- persistent pools (stack allocator: order matters) ----
        cpool = ctx.enter_context(tc.tile_pool(name="consts", bufs=1))
        xpool = ctx.enter_context(tc.tile_pool(name="xt", bufs=1))
        qkpool = ctx.enter_context(tc.tile_pool(name="qkT", bufs=1))
        vpool = ctx.enter_context(tc.tile_pool(name="vsb", bufs=1))
        apool = ctx.enter_context(tc.tile_pool(name="attnT", bufs=1))
        wopool = ctx.enter_context(tc.tile_pool(name="wo", bufs=1))
        wspool = ctx.enter_context(tc.tile_pool(name="wstream", bufs=6))
        workpool = ctx.enter_context(tc.tile_pool(name="work", bufs=1))

        cc = cpool.tile([128, S_], bf16, tag="cc", name="cc")
        ss = cpool.tile([128, S_], bf16, tag="ss", name="ss")
        tri = cpool.tile([128, 128], bf16, tag="tri", name="tri")

        # x tiles in d order on the two HWDGE queues: the split-contraction
        # slot-0 phases consume d 0..7 first, then 8..15.  Head-0 weights go
        # FIRST on these fast queues (the SWDGE queue crawls behind the x
        # load on HBM, and phase A is weight-blocked without them).
        xts = [xpool.tile([128, S_], bf16, tag=f"x{d}", name=f"x{d}")
               for d in range(DT)]
        DH = DT // 2               # phase-A contraction depth
        w0_tiles = {}
        for d in range(DT):
            eng = nc.sync if d % 2 == 0 else nc.scalar
            eng.dma_start(xts[d][:], xP_d[:, d * S_:(d + 1) * S_])


        qkT = [[qkpool.tile([128, S_], bf16, tag=f"qk{h}_{p}", name=f"qk{h}_{p}")
                for p in range(2)] for h in range(H_)]
        vsb = [vpool.tile([128, VW], bf16, tag=f"v{sb}", name=f"v{sb}")
               for sb in range(SBK)]
        for sb in range(SBK):       # ones columns for the fused row-sum
            nc.vector.memset(vsb[sb][:, 128::129], 1.0)

        attnT = [apool.tile([128, S_], bf16, tag=f"at{h}", name=f"at{h}")
                 for h in range(H_)]
        wot = [wopool.tile([128, Dm_], bf16, tag=f"wo{h}", name=f"wo{h}")
               for h in range(H_)]

        wq_tiles = dict(w0_tiles)

        def fetch_w(h):
            for p in (1, 0):    # k first: units consume k swaths first
                wt = wspool.tile([128, DT * 128], bf16, tag="wst", name=f"wqk{h}{p}")
                blk = 2 * h + p
                nc.gpsimd.dma_start(wt[:], wqkP_d[:, blk * DT * 128:(blk + 1) * DT * 128])
                wq_tiles[(h, p)] = wt
            wt = wspool.tile([128, DT * 128], bf16, tag="wst", name=f"wv{h}")
            nc.gpsimd.dma_start(wt[:], wvP_d[:, h * DT * 128:(h + 1) * DT * 128])
            wq_tiles[(h, "v")] = wt

        def fetch_wo(h):
            nc.gpsimd.dma_start(wot[h][:], woP_d[:, h * Dm_:(h + 1) * Dm_])

        def wtile(shape, dtype, tag, bufs):
            return workpool.tile(shape, dtype, tag=tag, name=tag, bufs=bufs)

        with tc.tile_pool(name="stps", bufs=3, space="PSUM") as stps, \
             tc.tile_pool(name="pvps", bufs=2, space="PSUM") as pvps:

            # ---------------- emitters ----------------
            def emit_v(h, sb, qkvps, d0=0, d1=None, half=None, tag="qkv"):
                d1 = DT if d1 is None else d1
                ps = qkvps.tile([128, 512], f32, tag=tag, name="psv")
                wt = wq_tiles[(h, "v")]
                for d in range(d0, d1):
                    nc.tensor.matmul(ps[:, 0:128], xts[d][:, sb * 128:(sb + 1) * 128],
                                     wt[:, d * 128:(d + 1) * 128],
                                     start=(d == d0), stop=(d == d1 - 1))
                if half is not None and d0 == 0:
                    nc.vector.tensor_copy(half, ps[:, 0:128])   # phase A spill
                    return
                if half is not None:
                    nc.vector.scalar_tensor_tensor(
                        vsb[sb][:, h * 129:h * 129 + 128], ps[:, 0:128], 0.0, half,
                        mybir.AluOpType.bypass, mybir.AluOpType.add)
                else:
                    nc.vector.tensor_copy(vsb[sb][:, h * 129:h * 129 + 128], ps[:, 0:128])

            def emit_qk(h, p, sw, qkvps, d0=0, d1=None, half=None, tag="qkv"):
                """One q/k projection unit.  With d1 set and half given,
                either spills the partial sum (phase A: d0==0) or combines
                psum + half and continues to rope (phase B)."""
                d1 = DT if d1 is None else d1
                ps = qkvps.tile([128, 512], f32, tag=tag, name="psqk")
                wt = wq_tiles[(h, p)]
                for d in range(d0, d1):
                    nc.tensor.matmul(ps[:], wt[:, d * 128:(d + 1) * 128],
                                     xts[d][:, sw * 512:(sw + 1) * 512],
                                     start=(d == d0), stop=(d == d1 - 1))
                if half is not None and d0 == 0:
                    # phase A spill on DVE: the Act sequencer is busy issuing
                    # x DMAs at startup and would hold the psum buffers
                    nc.vector.tensor_copy(half, ps[:])
                    return
                raw = wtile([128, 512], bf16, "raw", 3)
                if half is not None:
                    nc.vector.scalar_tensor_tensor(
                        raw[:], ps[:], 0.0, half,
                        mybir.AluOpType.bypass, mybir.AluOpType.add)
                else:
                    nc.scalar.copy(raw[:], ps[:])
                m2 = wtile([128, 512], bf16, "m2", 2)
                nc.vector.tensor_mul(m2[:], raw[:], ss[:, sw * 512:(sw + 1) * 512])
                m2s = wtile([128, 512], bf16, "m2s", 2)
                nc.vector.stream_shuffle(m2s[:], m2[:], mask=SWAP16)
                m1 = wtile([128, 512], bf16, "m1", 2)
                nc.vector.tensor_mul(m1[:], raw[:], cc[:, sw * 512:(sw + 1) * 512])
                nc.vector.tensor_add(qkT[h][p][:, sw * 512:(sw + 1) * 512], m1[:], m2s[:])

            def qkv_closures(h, cur_pool):
                """v and qk units for head h, interleaved v-between-qk.
                k (p=1) before q so the next head's attention never waits on
                the last-emitted rope chain.  cur_pool is read at CALL time
                so closures drained after qkvps closes can be redirected."""
                vs = [lambda h=h, sb=sb: emit_v(h, sb, cur_pool["pool"],
                                                tag=cur_pool["tag"])
                      for sb in range(SBK)]
                qs = [lambda h=h, p=p, sw=sw: emit_qk(h, p, sw, cur_pool["pool"],
                                                      tag=cur_pool["tag"])
                      for p in (1, 0) for sw in range(NSW)]
                mixed, vi, acc = [], 0, 0.0
                ratio = len(vs) / max(1, len(qs))
                for q in qs:
                    mixed.append(q)
                    acc += ratio
                    while acc >= 1.0 and vi < len(vs):
                        mixed.append(vs[vi]); vi += 1; acc -= 1.0
                mixed.extend(vs[vi:])
                return mixed

            def attn_head(h, filler, gate=None, eager_last=False, need=None):
                """Emit attention for head h.  `filler`: zero-arg closures
                drained between key blocks to keep the PE busy.  `gate(g)`
                limits how many fillers may run before group g completes
                (None = no gate).  st runs LOOKAHEAD key blocks ahead of pv
                so Act exp latency is hidden."""
                LOOKAHEAD = 2
                fi = [0]
                total_jb = sum(4 * g + 4 for g in range(NG))
                per_jb = len(filler) / max(1, total_jb)
                acc = [0.0]

                def drain(limit):
                    while acc[0] >= 1.0 and fi[0] < limit:
                        filler[fi[0]]()
                        fi[0] += 1
                        acc[0] -= 1.0

                for g in range(NG):
                    njb = 4 * g + 4
                    eager = eager_last and g == NG - 1
                    limit = len(filler) if gate is None else gate(g)
                    if need is not None:
                        while fi[0] < min(need[g], len(filler)):
                            filler[fi[0]]()
                            fi[0] += 1
                    pva = pvps.tile([128, 258], f32, tag="pv", name="pva")
                    pvb = pvps.tile([128, 258], f32, tag="pv", name="pvb")
                    slab = wtile([128, 512], bf16, "slab", 3)

                    def pvc(c):
                        t = pva if c < 2 else pvb
                        return t[:, (c % 2) * 129:(c % 2) * 129 + 129]

                    def norm_chunk(c):
                        rc = wtile([128, 1], f32, "rc", 4)
                        nc.vector.reciprocal(rc[:], pvc(c)[:, 128:129])
                        nc.vector.tensor_scalar_mul(slab[:, c * 128:(c + 1) * 128],
                                                    pvc(c)[:, 0:128], rc[:])

                    pend = []   # (jb, dgi, off, pt, ptm)

                    def emit_st(jb):
                        dgi = jb - 4 * g
                        off = 128 * dgi if dgi >= 0 else 0
                        width = 512 - off
                        st = stps.tile([128, 512], f32, tag="st", name="st")
                        nc.tensor.matmul(st[:, :width],
                                         qkT[h][1][:, jb * 128:(jb + 1) * 128],
                                         qkT[h][0][:, g * 512 + off:(g + 1) * 512],
                                         start=True, stop=True)
                        pt = wtile([128, 512], bf16, "pt", 4)
                        nc.scalar.activation(pt[:, :width], st[:, :width],
                                             AF.Exp, scale=SCALE)
                        ptm = None
                        if dgi >= 0:
                            ptm = wtile([128, 128], bf16, "ptm", 3)
                            nc.vector.tensor_mul(ptm[:], pt[:, 0:128], tri[:])
                        pend.append((jb, dgi, off, pt, ptm))

                    def emit_pv():
                        # two chunks share one psum bank: only the bank's
                        # first matmul may set start (it pending-zeroes the
                        # whole 2KB zero region) and only its last sets stop
                        jb, dgi, off, pt, ptm = pend.pop(0)
                        for c in range(max(0, dgi), 4):
                            lhsT = ptm[:] if (dgi >= 0 and c == dgi) \
                                else pt[:, (c * 128 - off):(c * 128 - off) + 128]
                            nc.tensor.matmul(pvc(c), lhsT,
                                             vsb[jb][:, h * 129:(h + 1) * 129],
                                             start=(jb == 0 and c % 2 == 0),
                                             stop=(c % 2 == 1 and jb == 4 * g + c),
                                             skip_group_check=True)
                        if eager and dgi >= 0:
                            # chunk dgi just finished: normalize + transpose it
                            # now so o_proj of this group can start before the
                            # group ends
                            norm_chunk(dgi)
                            nc.sync.dma_start_transpose(
                                attnT[h][:, g * 512 + dgi * 128:g * 512 + (dgi + 1) * 128],
                                slab[:, dgi * 128:(dgi + 1) * 128])

                    for jb in range(njb):
                        emit_st(jb)
                        acc[0] += per_jb
                        drain(limit)
                        if len(pend) > LOOKAHEAD:
                            emit_pv()
                    while pend:
                        emit_pv()
                    if not eager:
                        # normalize group g -> attn slab [i, d] -> DMA transpose
                        for c in range(4):
                            norm_chunk(c)
                        teng = nc.sync if gate is not None else nc.scalar
                        teng.dma_start_transpose(
                            attnT[h][:, g * 512:(g + 1) * 512].rearrange("p (c f) -> p c f", c=4),
                            slab[:])
                    if gate is None:
                        # boundary filler covers the norm->next-group psum WAR
                        acc[0] = max(acc[0], 1.0)
                        drain(limit)
                # flush remaining fillers
                acc[0] = float(len(filler))
                drain(len(filler))

            # ---------------- schedule ----------------
            with tc.tile_pool(name="qkvps", bufs=2, space="PSUM") as qkvps:
                fetch_w(0)
                if H_ > 1:
                    fetch_w(1)
                # rope tables / mask after the weights: not needed until the
                # first qk psum closes, so keep HBM free for the x/w load
                nc.gpsimd.dma_start(cc[:], cc_dram[:])
                nc.gpsimd.dma_start(ss[:], ss_dram[:])
                nc.gpsimd.dma_start(tri[:], tri_dram[:])
                # slot 0: qkv(h0), contraction split in two halves so the PE
                # can work while the second half of x is still loading; the
                # idle attention st banks give 5-way psum concurrency so
                # progress tracks x-slice arrival
                qkhalf = [wtile([128, Dm_], bf16, "ostage", 2) for _ in range(2)]
                vhalf = wtile([128, SBK * 128], bf16, "vhalf", 1)
                qk_units = [(p, sw) for p in (1, 0) for sw in range(NSW)]
                pools = [(qkvps, "qkv"), (stps, "st"), (qkvps, "qkv"),
                         (stps, "st"), (stps, "st")]

                def qk_half(i):
                    return qkhalf[i // 4][:, (i % 4) * 512:(i % 4 + 1) * 512]

                # phase A: first contraction half of every unit, 5-way psum
                for i, (p, sw) in enumerate(qk_units):
                    pool, ptag = pools[i % len(pools)]
                    emit_qk(0, p, sw, pool, d0=0, d1=DH, tag=ptag, half=qk_half(i))
                for sb in range(SBK):
                    pool, ptag = pools[sb % len(pools)]
                    emit_v(0, sb, pool, d0=0, d1=DH, tag=ptag,
                           half=vhalf[:, sb * 128:(sb + 1) * 128])

                # phase B closures (second half + rope), per swath group so
                # attn(h0) can start as soon as its swaths are done
                def pb_qk(i):
                    p, sw = qk_units[i]
                    return lambda: emit_qk(0, p, sw, qkvps, d0=DH, d1=DT,
                                           half=qk_half(i))

                def pb_v(sb):
                    return lambda: emit_v(0, sb, qkvps, d0=DH, d1=DT,
                                          half=vhalf[:, sb * 128:(sb + 1) * 128])

                # prefix: k sw0, q sw0, v sb0..3 -> unblocks attn(h0) g0
                nv0 = min(4, SBK)
                pb_qk(0)()
                pb_qk(NSW)()
                for sb in range(nv0):
                    pb_v(sb)()
                rest = []
                needs = [0]
                for sw in range(1, NSW):
                    rest.append(pb_qk(sw))            # k sw
                    rest.append(pb_qk(NSW + sw))      # q sw
                    for sb in range(4 * sw, min(4 * (sw + 1), SBK)):
                        rest.append(pb_v(sb))
                    needs.append(len(rest))
                while len(needs) < NG:
                    needs.append(len(rest))
                reserved = []                      # last-head qkv units held
                cur_pool = {"pool": qkvps, "tag": "qkv"}
                if H_ == 1:
                    for u in rest:
                        u()
                    for hh in range(H_):
                        fetch_wo(hh)
                else:
                    # attn(h0) interleaves the rest of phase B, then h1's
                    # units; w(h2) is fetched as a filler so its buffer
                    # rotation follows all w(h0) readers in emission order
                    h1_units = qkv_closures(1, cur_pool)
                    mid = [] if H_ <= 2 else [lambda: fetch_w(2)]
                    attn_head(0, rest + mid + h1_units, need=needs)
                    for h in range(2, H_):         # slots 2..H-1
                        if h + 1 < H_:
                            fetch_w(h + 1)
                        if h == H_ - 1:
                            for hh in range(H_):
                                fetch_wo(hh)
                        units = qkv_closures(h, cur_pool)
                        if h == H_ - 1 and len(units) > 2:
                            reserved = units[-2:]  # fillers for attn(h3) g0/g1
                            units = units[:-2]
                        attn_head(h - 1, units)
                    if H_ == 2:
                        for hh in range(H_):
                            fetch_wo(hh)

            with tc.tile_pool(name="opsps", bufs=2, space="PSUM") as opsps:
                # alternate stage-copy engines: all-on-one-queue delays the
                # latency-critical exp (Act) / norm (DVE) chains of attn(h3)
                eng_rr = [nc.scalar.copy, nc.vector.tensor_copy]

                def oproj_unit(sb):
                    stage = wtile([128, Dm_], bf16, "ostage", 2)
                    tail = sb >= SBK - 2
                    for db in range(NDB):
                        ps = opsps.tile([128, 512], f32, tag="ops", name="ops")
                        for hh in range(H_):
                            nc.tensor.matmul(ps[:],
                                             attnT[hh][:, sb * 128:(sb + 1) * 128],
                                             wot[hh][:, db * 512:(db + 1) * 512],
                                             start=(hh == 0), stop=(hh == H_ - 1))
                        copy = eng_rr[(sb * NDB + db) % len(eng_rr)]
                        copy(stage[:, db * 512:(db + 1) * 512], ps[:])
                        if tail:    # drain the last rows in db chunks on both
                            eng = nc.sync if db % 2 == 0 else nc.scalar
                            eng.dma_start(
                                out_d[sb * 128:(sb + 1) * 128, db * 512:(db + 1) * 512],
                                stage[:, db * 512:(db + 1) * 512])
                    if not tail:
                        nc.sync.dma_start(out_d[sb * 128:(sb + 1) * 128, :], stage[:])

                # last head's attention: two reserved qkv units fill the first
                # groups (redirected to the st psum pool now that qkvps is
                # closed), then o_proj units gated per finished group; the
                # last group norms/transposes per chunk so its o_proj units
                # unblock before the group ends
                cur_pool["pool"], cur_pool["tag"] = stps, "st"
                nres = len(reserved)
                filler = reserved + [lambda sb=sb: oproj_unit(sb) for sb in range(SBK)]
                attn_head(H_ - 1, filler, gate=lambda g: nres + 4 * g,
                          eager_last=True)

    nc.compile()
    return nc


# --------------------------------------------------------------------------
# host sharding + entry point
# --------------------------------------------------------------------------

def _prep_core_inputs(x, Wq, Wk, Wv, Wo, fp32r=None):
    """Return list of 8 per-core input dicts (bf16, pre-tiled [128, ...])."""
    import ml_dtypes
    bf = ml_dtypes.bfloat16
    perm = _deinterleave_idx()
    DT = D // 128
    in_maps = []
    for c in range(N_CORES):
        b, g = c // GROUPS, c % GROUPS
        heads = range(g * H_PER_CORE, (g + 1) * H_PER_CORE)
        qk_rows = np.concatenate([h * HD + perm for h in heads])
        v_rows = np.concatenate([np.arange(h * HD, (h + 1) * HD) for h in heads])
        wq_t = Wq[qk_rows, :].T.astype(np.float32)   # [D, E]
        wk_t = Wk[qk_rows, :].T.astype(np.float32)
        wv_t = Wv[v_rows, :].T.astype(np.float32)
        wo_t = Wo[:, v_rows].T.astype(np.float32)    # [E, D]

        # x tiles, d-major: xP[p, d*S+s] = x[b][s, d*128+p]
        xb = np.ascontiguousarray(x[b]).astype(bf)               # [S, D]
        xP = xb.T.reshape(DT, 128, S).transpose(1, 0, 2).reshape(128, DT * S)

        def tile_w(w):
            """w [D, C] -> [128, (C/128)*DT*128]: per 128-col block, DT
            d-tiles of [128, 128] laid out d-major."""
            C = w.shape[1]
            nb = C // 128
            out = np.empty((128, nb * DT * 128), dtype=bf)
            wb = w.astype(bf)
            for bi in range(nb):
                t3 = wb[:, bi * 128:(bi + 1) * 128].reshape(DT, 128, 128)
                out[:, bi * DT * 128:(bi + 1) * DT * 128] = (
                    t3.transpose(1, 0, 2).reshape(128, DT * 128))
            return out

        # wqk blocks in (h, p) order: block 2h = q head h, block 2h+1 = k
        wqk = np.empty((D, 2 * E_PER_CORE), dtype=np.float32)
        for h in range(H_PER_CORE):
            wqk[:, (2 * h) * 128:(2 * h + 1) * 128] = wq_t[:, h * 128:(h + 1) * 128]
            wqk[:, (2 * h + 1) * 128:(2 * h + 2) * 128] = wk_t[:, h * 128:(h + 1) * 128]

        woP = wo_t.reshape(H_PER_CORE, 128, D).transpose(1, 0, 2).reshape(
            128, H_PER_CORE * D).astype(bf)

        in_maps.append({
            "xP": np.ascontiguousarray(xP),
            "wqkP": np.ascontiguousarray(tile_w(wqk)),
            "wvP": np.ascontiguousarray(tile_w(wv_t)),
            "woP": np.ascontiguousarray(woP),
        })
    return in_maps


def _is_causal_mask(attn_mask):
    if attn_mask is None:
        return True
    m = np.asarray(attn_mask)
    if m.shape != (1, 1, S, S):
        return False
    m2 = m[0, 0]
    tril = np.tril(np.ones((S, S), dtype=bool))
    return bool(np.all(m2[tril] == 0.0) and np.all(m2[~tril] <= -1.0e30))


def _get_program(mmdt="bf16"):
    key = ("full", "bf16")
    if key not in _CACHE:
        _CACHE[key] = build(S, D, H_PER_CORE)
    return _CACHE[key]


def run_on_hw(in_maps, mmdt="bf16", trace=False, **kwargs):
    """Run the SPMD program on the 8 NeuronCores; returns BassKernelResults."""
    from concourse.bass_utils import run_bass_kernel_spmd
    nc = _get_program(mmdt)
    return run_bass_kernel_spmd(nc, in_maps, core_ids=list(range(N_CORES)),
                                trace=trace, **kwargs)


def kernel(x, Wq, Wk, Wv, Wo, attn_mask=None, **_ignored):
    x = np.asarray(x, dtype=np.float32)
    Wq = np.asarray(Wq, dtype=np.float32)
    Wk = np.asarray(Wk, dtype=np.float32)
    Wv = np.asarray(Wv, dtype=np.float32)
    Wo = np.asarray(Wo, dtype=np.float32)

    if not _is_causal_mask(attn_mask):
        return _np_reference(x, Wq, Wk, Wv, Wo,
                             np.asarray(attn_mask, dtype=np.float32)).astype(np.float32)

    in_maps = _prep_core_inputs(x, Wq, Wk, Wv, Wo)
    res = run_on_hw(in_maps, trace=False)

    out = np.zeros((B, S, D), dtype=np.float32)
    for c in range(N_CORES):
        out[c // GROUPS] += res.results[c]["out"].astype(np.float32)
    return out


# revision 52
# speedup vs baseline: 1.0302x; 1.0302x over previous
"""Trainium2 Bass kernel: causal self-attention with RoPE (16 heads, B=2, S=2048, D=2048).

Sharding: 8 cores = 2 (batch, data-parallel) x 4 (head-groups of 4 heads, tensor
parallel).  Each core computes q/k/v projections for its 4 heads, RoPE, causal
attention, and a partial o_proj over its 512 rows of Wo.  The 4 partial [S, D]
outputs per batch are summed on the host (the "all-reduce" of o_proj).

All matmuls run in bf16 (cast on host), psum accumulation in f32.  Structure:
  - single x window resident in SBUF (bf16), weights streamed once
  - softmax denominator fused into the PV matmul: pt (exp scores) is the
    stationary operand, moving operand is [v | 1] (ones column baked into the
    v SBUF layout at stride 129), so the row-sum lands in psum col 128 free
  - causal diag blocks compute only the valid suffix; triangular boundary
    chunks masked post-exp on DVE
  - attention output [i, d] flipped to [d, i] via DMA-XBAR transpose
  - head-major software pipeline: attn(h) interleaved with qkv(h+1) on the
    PE queue so the tensor engine never waits on Act exp; st runs two key
    blocks ahead of pv
"""

import math

import numpy as np

# ---- problem constants ----
B, S, D = 2, 2048, 2048
NUM_HEADS, HD = 16, 128
N_CORES = 8
GROUPS = 4                  # head-groups (tensor-parallel)
H_PER_CORE = NUM_HEADS // GROUPS   # 4
E_PER_CORE = H_PER_CORE * HD       # 512

_CACHE = {}


# --------------------------------------------------------------------------
# host-side helpers
# --------------------------------------------------------------------------

def _rope_sin_cos(seq_len, head_dim):
    """float32, matches reference._rope_sin_cos."""
    pos = np.arange(seq_len, dtype=np.float32)
    freq_seq = np.arange(0, head_dim, 2, dtype=np.float32)
    inv_freq = (np.float32(1.0) / (np.float32(10000.0) ** (freq_seq / np.float32(head_dim)))).astype(np.float32)
    sinusoid = pos[:, None] * inv_freq[None, :]          # [S, hd/2]
    return np.sin(sinusoid).astype(np.float32), np.cos(sinusoid).astype(np.float32)


def _rope_tables(seq_len):
    """CC / SS' [128, seq_len] f32 in the quadrant-paired layout.
    CC row = cos(pair angle) at both x1 and x2 rows.
    SS' = +sin at x1 rows, -sin at x2 rows, so that
    shuffle16(ps*SS') = [-x2*sin at x1 rows ; x1*sin at x2 rows]."""
    sin, cos = _rope_sin_cos(seq_len, HD)       # [S, 64]
    cosT = cos.T                                # [64, S] pair-index order
    sinT = sin.T
    x1, x2 = _pair_pos()
    CC = np.empty((HD, seq_len), dtype=np.float32)
    SS = np.empty((HD, seq_len), dtype=np.float32)
    CC[x1] = cosT
    CC[x2] = cosT
    SS[x1] = sinT
    SS[x2] = -sinT
    return CC, SS


def _deinterleave_idx():
    """Row permutation within one head: quadrant-paired so the RoPE partner
    swap is a within-quadrant rotation by 16 (DVE stream_shuffle-able)."""
    idx = np.empty(HD, dtype=np.int64)
    for q in range(4):
        t = 16 * q + np.arange(16)
        idx[q * 32:q * 32 + 16] = 2 * t
        idx[q * 32 + 16:q * 32 + 32] = 2 * t + 1
    return idx


def _pair_pos():
    """(x1_rows, x2_rows) in the deinterleaved layout, pair-index order."""
    x1 = np.concatenate([q * 32 + np.arange(16) for q in range(4)])
    x2 = x1 + 16
    return x1, x2


def _np_rope_apply(q, sin, cos):
    """q: [S, 128] in the quadrant-paired deinterleaved layout."""
    p1, p2 = _pair_pos()
    x1, x2 = q[:, p1], q[:, p2]
    r = np.empty_like(q)
    r[:, p1] = x1 * cos - x2 * sin
    r[:, p2] = x1 * sin + x2 * cos
    return r


def _np_core_model(xT, wq, wk, wv, wo):
    """Numpy model of what ONE core's device program computes (f32 version
    of the math; device uses bf16)."""
    Dm, S_ = xT.shape
    E_ = wq.shape[1]
    H_ = E_ // HD
    x = xT.T.astype(np.float32)
    sin, cos = _rope_sin_cos(S_, HD)
    out = np.zeros((S_, Dm), dtype=np.float32)
    causal = np.tril(np.ones((S_, S_), dtype=bool))
    for h in range(H_):
        q = x @ wq[:, h * HD:(h + 1) * HD]
        k = x @ wk[:, h * HD:(h + 1) * HD]
        v = x @ wv[:, h * HD:(h + 1) * HD]
        q = _np_rope_apply(q, sin, cos)
        k = _np_rope_apply(k, sin, cos)
        s = (q @ k.T) / math.sqrt(HD)
        s = np.where(causal, s, -np.inf)
        p = np.exp(s - s.max(axis=-1, keepdims=True))
        p = p / p.sum(axis=-1, keepdims=True)
        out += (p @ v) @ wo[h * HD:(h + 1) * HD, :]
    return out


def _np_reference(x, Wq, Wk, Wv, Wo, attn_mask):
    """Full-problem numpy fallback replicating reference.py (generic mask)."""
    B_, S_, D_ = x.shape
    H = NUM_HEADS
    hd = D_ // H
    sin, cos = _rope_sin_cos(S_, hd)

    def proj(W):
        y = np.einsum('bsd,ed->bse', x, W)
        return y.reshape(B_, S_, H, hd).transpose(0, 2, 1, 3)

    q, k, v = proj(Wq), proj(Wk), proj(Wv)

    def rope(t):
        tr = t.reshape(B_, H, S_, hd // 2, 2)
        x1, x2 = tr[..., 0], tr[..., 1]
        r1 = x1 * cos[None, None] - x2 * sin[None, None]
        r2 = x1 * sin[None, None] + x2 * cos[None, None]
        return np.stack((r1, r2), axis=-1).reshape(B_, H, S_, hd)

    q, k = rope(q), rope(k)
    scores = np.einsum('bhqd,bhkd->bhqk', q, k) / math.sqrt(hd) + attn_mask
    scores = scores - scores.max(axis=-1, keepdims=True)
    p = np.exp(scores)
    p = p / p.sum(axis=-1, keepdims=True)
    attn = np.einsum('bhqk,bhkd->bhqd', p, v)
    attn = attn.transpose(0, 2, 1, 3).reshape(B_, S_, D_)
    return np.einsum('bsd,ed->bse', attn, Wo)


# --------------------------------------------------------------------------
# device program builder
# --------------------------------------------------------------------------

def build(S_=S, Dm_=D, H_=H_PER_CORE, mmdt="bf16"):
    """Build the per-core Bass program (SPMD: same program, 8 data shards).

    Inputs (DRAM, bf16, host pre-tiled to [128, ...] partition-major):
      xP   [128, DT*S_]        x tiles, d-major (xP[p, d*S_+s] = x[s, d*128+p])
      wqkP [128, 2*H_*DT*128]  per (h,q|k) block of DT d-tiles of [128,128]
      wvP  [128, H_*DT*128]    per h block of DT d-tiles
      woP  [128, H_*Dm_]       woP[p, h*Dm_+j] = Wo_core[h*128+p, j]
    Output (DRAM): out [S_, Dm_] bf16 (partial o_proj; host sums groups)
    """
    import concourse.tile as tile
    from concourse import bacc, mybir
    import ml_dtypes

    f32 = mybir.dt.float32
    bf16 = mybir.dt.bfloat16
    AF = mybir.ActivationFunctionType

    DT = Dm_ // 128            # contraction tiles
    NG = S_ // 512             # attention i-groups
    SBK = S_ // 128            # s-blocks
    NDB = Dm_ // 512           # o_proj D chunks
    NSW = S_ // 512            # qk projection s-swaths
    SCALE = 1.0 / math.sqrt(HD)
    SWAP16 = [(i + 16) % 32 for i in range(32)]
    VW = H_ * 129              # v sbuf tile width (ones col per head)

    nc = bacc.Bacc("TRN2", target_bir_lowering=False, debug=False)

    xP_d = nc.dram_tensor("xP", [128, DT * S_], bf16, kind="ExternalInput")
    wqkP_d = nc.dram_tensor("wqkP", [128, 2 * H_ * DT * 128], bf16, kind="ExternalInput")
    wvP_d = nc.dram_tensor("wvP", [128, H_ * DT * 128], bf16, kind="ExternalInput")
    woP_d = nc.dram_tensor("woP", [128, H_ * Dm_], bf16, kind="ExternalInput")
    out_d = nc.dram_tensor("out", [S_, Dm_], bf16, kind="ExternalOutput")

    CC_np, SS_np = _rope_tables(S_)
    cc_dram = nc.inline_tensor(CC_np.astype(ml_dtypes.bfloat16), "cc_const")
    ss_dram = nc.inline_tensor(SS_np.astype(ml_dtypes.bfloat16), "ss_const")
    tri_np = np.triu(np.ones((128, 128), dtype=np.float32)).astype(ml_dtypes.bfloat16)
    tri_dram = nc.inline_tensor(tri_np, "tri_const")

    from contextlib import ExitStack

    with tile.TileContext(nc) as tc, ExitStack() as ctx:
        # ---- persistent pools (stack allocator: order matters) ----
        cpool = ctx.enter_context(tc.tile_pool(name="consts", bufs=1))
        xpool = ctx.enter_context(tc.tile_pool(name="xt", bufs=1))
        qkpool = ctx.enter_context(tc.tile_pool(name="qkT", bufs=1))
        vpool = ctx.enter_context(tc.tile_pool(name="vsb", bufs=1))
        apool = ctx.enter_context(tc.tile_pool(name="attnT", bufs=1))
        wopool = ctx.enter_context(tc.tile_pool(name="wo", bufs=1))
        wspool = ctx.enter_context(tc.tile_pool(name="wstream", bufs=6))
        workpool = ctx.enter_context(tc.tile_pool(name="work", bufs=1))

        cc = cpool.tile([128, S_], bf16, tag="cc", name="cc")
        ss = cpool.tile([128, S_], bf16, tag="ss", name="ss")
        tri = cpool.tile([128, 128], bf16, tag="tri", name="tri")

        # x tiles in d order on the two HWDGE queues: the split-contraction
        # slot-0 phases consume d 0..7 first, then 8..15.  Head-0 weights go
        # FIRST on these fast queues (the SWDGE queue crawls behind the x
        # load on HBM, and phase A is weight-blocked without them).
        xts = [xpool.tile([128, S_], bf16, tag=f"x{d}", name=f"x{d}")
               for d in range(DT)]
        DH = DT // 2               # phase-A contraction depth
        w0_tiles = {}
        for d in range(DT):
            eng = nc.sync if d % 2 == 0 else nc.scalar
            eng.dma_start(xts[d][:], xP_d[:, d * S_:(d + 1) * S_])


        qkT = [[qkpool.tile([128, S_], bf16, tag=f"qk{h}_{p}", name=f"qk{h}_{p}")
                for p in range(2)] for h in range(H_)]
        vsb = [vpool.tile([128, VW], bf16, tag=f"v{sb}", name=f"v{sb}")
               for sb in range(SBK)]
        for sb in range(SBK):       # ones columns for the fused row-sum
            nc.vector.memset(vsb[sb][:, 128::129], 1.0)

        attnT = [apool.tile([128, S_], bf16, tag=f"at{h}", name=f"at{h}")
                 for h in range(H_)]
        wot = [wopool.tile([128, Dm_], bf16, tag=f"wo{h}", name=f"wo{h}")
               for h in range(H_)]

        wq_tiles = dict(w0_tiles)

        def fetch_w(h):
            for p in (1, 0):    # k first: units consume k swaths first
                wt = wspool.tile([128, DT * 128], bf16, tag="wst", name=f"wqk{h}{p}")
                blk = 2 * h + p
                nc.gpsimd.dma_start(wt[:], wqkP_d[:, blk * DT * 128:(blk + 1) * DT * 128])
                wq_tiles[(h, p)] = wt
            wt = wspool.tile([128, DT * 128], bf16, tag="wst", name=f"wv{h}")
            nc.gpsimd.dma_start(wt[:], wvP_d[:, h * DT * 128:(h + 1) * DT * 128])
            wq_tiles[(h, "v")] = wt

        def fetch_wo(h):
            nc.gpsimd.dma_start(wot[h][:], woP_d[:, h * Dm_:(h + 1) * Dm_])

        def wtile(shape, dtype, tag, bufs):
            return workpool.tile(shape, dtype, tag=tag, name=tag, bufs=bufs)

        with tc.tile_pool(name="stps", bufs=3, space="PSUM") as stps, \
             tc.tile_pool(name="pvps", bufs=2, space="PSUM") as pvps:

            # ---------------- emitters ----------------
            def emit_v(h, sb, qkvps, d0=0, d1=None, half=None, tag="qkv"):
                d1 = DT if d1 is None else d1
                ps = qkvps.tile([128, 512], f32, tag=tag, name="psv")
                wt = wq_tiles[(h, "v")]
                for d in range(d0, d1):
                    nc.tensor.matmul(ps[:, 0:128], xts[d][:, sb * 128:(sb + 1) * 128],
                                     wt[:, d * 128:(d + 1) * 128],
                                     start=(d == d0), stop=(d == d1 - 1))
                if half is not None and d0 == 0:
                    nc.vector.tensor_copy(half, ps[:, 0:128])   # phase A spill
                    return
                if half is not None:
                    nc.vector.scalar_tensor_tensor(
                        vsb[sb][:, h * 129:h * 129 + 128], ps[:, 0:128], 0.0, half,
                        mybir.AluOpType.bypass, mybir.AluOpType.add)
                else:
                    nc.vector.tensor_copy(vsb[sb][:, h * 129:h * 129 + 128], ps[:, 0:128])

            def emit_qk(h, p, sw, qkvps, d0=0, d1=None, half=None, tag="qkv"):
                """One q/k projection unit.  With d1 set and half given,
                either spills the partial sum (phase A: d0==0) or combines
                psum + half and continues to rope (phase B)."""
                d1 = DT if d1 is None else d1
                ps = qkvps.tile([128, 512], f32, tag=tag, name="psqk")
                wt = wq_tiles[(h, p)]
                for d in range(d0, d1):
                    nc.tensor.matmul(ps[:], wt[:, d * 128:(d + 1) * 128],
                                     xts[d][:, sw * 512:(sw + 1) * 512],
                                     start=(d == d0), stop=(d == d1 - 1))
                if half is not None and d0 == 0:
                    # phase A spill on DVE: the Act sequencer is busy issuing
                    # x DMAs at startup and would hold the psum buffers
                    nc.vector.tensor_copy(half, ps[:])
                    return
                raw = wtile([128, 512], bf16, "raw", 3)
                if half is not None:
                    nc.vector.scalar_tensor_tensor(
                        raw[:], ps[:], 0.0, half,
                        mybir.AluOpType.bypass, mybir.AluOpType.add)
                else:
                    nc.scalar.copy(raw[:], ps[:])
                m2 = wtile([128, 512], bf16, "m2", 2)
                nc.vector.tensor_mul(m2[:], raw[:], ss[:, sw * 512:(sw + 1) * 512])
                m2s = wtile([128, 512], bf16, "m2s", 2)
                nc.vector.stream_shuffle(m2s[:], m2[:], mask=SWAP16)
                m1 = wtile([128, 512], bf16, "m1", 2)
                nc.vector.tensor_mul(m1[:], raw[:], cc[:, sw * 512:(sw + 1) * 512])
                nc.vector.tensor_add(qkT[h][p][:, sw * 512:(sw + 1) * 512], m1[:], m2s[:])

            def qkv_closures(h, cur_pool):
                """v and qk units for head h, interleaved v-between-qk.
                k (p=1) before q so the next head's attention never waits on
                the last-emitted rope chain.  cur_pool is read at CALL time
                so closures drained after qkvps closes can be redirected."""
                vs = [lambda h=h, sb=sb: emit_v(h, sb, cur_pool["pool"],
                                                tag=cur_pool["tag"])
                      for sb in range(SBK)]
                qs = [lambda h=h, p=p, sw=sw: emit_qk(h, p, sw, cur_pool["pool"],
                                                      tag=cur_pool["tag"])
                      for p in (1, 0) for sw in range(NSW)]
                mixed, vi, acc = [], 0, 0.0
                ratio = len(vs) / max(1, len(qs))
                for q in qs:
                    mixed.append(q)
                    acc += ratio
                    while acc >= 1.0 and vi < len(vs):
                        mixed.append(vs[vi]); vi += 1; acc -= 1.0
                mixed.extend(vs[vi:])
                return mixed

            def attn_head(h, filler, gate=None, eager_last=False, need=None):
                """Emit attention for head h.  `filler`: zero-arg closures
                drained between key blocks to keep the PE busy.  `gate(g)`
                limits how many fillers may run before group g completes
                (None = no gate).  st runs LOOKAHEAD key blocks ahead of pv
                so Act exp latency is hidden."""
                LOOKAHEAD = 2
                fi = [0]
                total_jb = sum(4 * g + 4 for g in range(NG))
                # front-loaded: fillers exhaust ~85% through, so the
                # last jbs never dump unconsumed rope work at the boundary
                per_jb = 1.15 * len(filler) / max(1, total_jb)
                acc = [0.0]

                def drain(limit):
                    while acc[0] >= 1.0 and fi[0] < limit:
                        filler[fi[0]]()
                        fi[0] += 1
                        acc[0] -= 1.0

                for g in range(NG):
                    njb = 4 * g + 4
                    eager = eager_last and g == NG - 1
                    limit = len(filler) if gate is None else gate(g)
                    if need is not None:
                        while fi[0] < min(need[g], len(filler)):
                            filler[fi[0]]()
                            fi[0] += 1
                    pva = pvps.tile([128, 258], f32, tag="pv", name="pva")
                    pvb = pvps.tile([128, 258], f32, tag="pv", name="pvb")
                    slab = wtile([128, 512], bf16, "slab", 3)

                    def pvc(c):
                        t = pva if c < 2 else pvb
                        return t[:, (c % 2) * 129:(c % 2) * 129 + 129]

                    def norm_chunk(c):
                        rc = wtile([128, 1], f32, "rc", 4)
                        nc.vector.reciprocal(rc[:], pvc(c)[:, 128:129])
                        nc.vector.tensor_scalar_mul(slab[:, c * 128:(c + 1) * 128],
                                                    pvc(c)[:, 0:128], rc[:])

                    pend = []   # (jb, dgi, off, pt, ptm)

                    def emit_st(jb):
                        dgi = jb - 4 * g
                        off = 128 * dgi if dgi >= 0 else 0
                        width = 512 - off
                        st = stps.tile([128, 512], f32, tag="st", name="st")
                        nc.tensor.matmul(st[:, :width],
                                         qkT[h][1][:, jb * 128:(jb + 1) * 128],
                                         qkT[h][0][:, g * 512 + off:(g + 1) * 512],
                                         start=True, stop=True)
                        pt = wtile([128, 512], bf16, "pt", 4)
                        nc.scalar.activation(pt[:, :width], st[:, :width],
                                             AF.Exp, scale=SCALE)
                        ptm = None
                        if dgi >= 0:
                            ptm = wtile([128, 128], bf16, "ptm", 3)
                            nc.vector.tensor_mul(ptm[:], pt[:, 0:128], tri[:])
                        pend.append((jb, dgi, off, pt, ptm))

                    def emit_pv():
                        # two chunks share one psum bank: only the bank's
                        # first matmul may set start (it pending-zeroes the
                        # whole 2KB zero region) and only its last sets stop
                        jb, dgi, off, pt, ptm = pend.pop(0)
                        for c in range(max(0, dgi), 4):
                            lhsT = ptm[:] if (dgi >= 0 and c == dgi) \
                                else pt[:, (c * 128 - off):(c * 128 - off) + 128]
                            nc.tensor.matmul(pvc(c), lhsT,
                                             vsb[jb][:, h * 129:(h + 1) * 129],
                                             start=(jb == 0 and c % 2 == 0),
                                             stop=(c % 2 == 1 and jb == 4 * g + c),
                                             skip_group_check=True)
                        if eager and dgi >= 0:
                            # chunk dgi just finished: normalize + transpose it
                            # now so o_proj of this group can start before the
                            # group ends
                            norm_chunk(dgi)
                            nc.sync.dma_start_transpose(
                                attnT[h][:, g * 512 + dgi * 128:g * 512 + (dgi + 1) * 128],
                                slab[:, dgi * 128:(dgi + 1) * 128])

                    for jb in range(njb):
                        emit_st(jb)
                        acc[0] += per_jb
                        drain(limit)
                        if len(pend) > LOOKAHEAD:
                            emit_pv()
                    while pend:
                        emit_pv()
                    if not eager:
                        # normalize group g -> attn slab [i, d] -> DMA transpose
                        for c in range(4):
                            norm_chunk(c)
                        teng = nc.sync if gate is not None else nc.scalar
                        teng.dma_start_transpose(
                            attnT[h][:, g * 512:(g + 1) * 512].rearrange("p (c f) -> p c f", c=4),
                            slab[:])
                    if gate is None:
                        # boundary filler covers the norm->next-group psum WAR
                        acc[0] = max(acc[0], 1.0)
                        drain(limit)
                # flush remaining fillers
                acc[0] = float(len(filler))
                drain(len(filler))

            # ---------------- schedule ----------------
            with tc.tile_pool(name="qkvps", bufs=2, space="PSUM") as qkvps:
                fetch_w(0)
                if H_ > 1:
                    fetch_w(1)
                # rope tables / mask after the weights: not needed until the
                # first qk psum closes, so keep HBM free for the x/w load
                nc.gpsimd.dma_start(cc[:], cc_dram[:])
                nc.gpsimd.dma_start(ss[:], ss_dram[:])
                nc.gpsimd.dma_start(tri[:], tri_dram[:])
                # slot 0: qkv(h0), contraction split in two halves so the PE
                # can work while the second half of x is still loading; the
                # idle attention st banks give 5-way psum concurrency so
                # progress tracks x-slice arrival
                qkhalf = [wtile([128, Dm_], bf16, "ostage", 2) for _ in range(2)]
                vhalf = wtile([128, SBK * 128], bf16, "vhalf", 1)
                qk_units = [(p, sw) for p in (1, 0) for sw in range(NSW)]
                pools = [(qkvps, "qkv"), (stps, "st"), (qkvps, "qkv"),
                         (stps, "st"), (stps, "st")]

                def qk_half(i):
                    return qkhalf[i // 4][:, (i % 4) * 512:(i % 4 + 1) * 512]

                # phase A: first contraction half of every unit, 5-way psum
                for i, (p, sw) in enumerate(qk_units):
                    pool, ptag = pools[i % len(pools)]
                    emit_qk(0, p, sw, pool, d0=0, d1=DH, tag=ptag, half=qk_half(i))
                for sb in range(SBK):
                    pool, ptag = pools[sb % len(pools)]
                    emit_v(0, sb, pool, d0=0, d1=DH, tag=ptag,
                           half=vhalf[:, sb * 128:(sb + 1) * 128])

                # phase B closures (second half + rope), per swath group so
                # attn(h0) can start as soon as its swaths are done
                def pb_qk(i):
                    p, sw = qk_units[i]
                    return lambda: emit_qk(0, p, sw, qkvps, d0=DH, d1=DT,
                                           half=qk_half(i))

                def pb_v(sb):
                    return lambda: emit_v(0, sb, qkvps, d0=DH, d1=DT,
                                          half=vhalf[:, sb * 128:(sb + 1) * 128])

                # prefix: k sw0, q sw0, v sb0..3 -> unblocks attn(h0) g0
                nv0 = min(4, SBK)
                pb_qk(0)()
                pb_qk(NSW)()
                for sb in range(nv0):
                    pb_v(sb)()
                rest = []
                needs = [0]
                for sw in range(1, NSW):
                    rest.append(pb_qk(sw))            # k sw
                    rest.append(pb_qk(NSW + sw))      # q sw
                    for sb in range(4 * sw, min(4 * (sw + 1), SBK)):
                        rest.append(pb_v(sb))
                    needs.append(len(rest))
                while len(needs) < NG:
                    needs.append(len(rest))
                reserved = []                      # last-head qkv units held
                cur_pool = {"pool": qkvps, "tag": "qkv"}
                if H_ == 1:
                    for u in rest:
                        u()
                    for hh in range(H_):
                        fetch_wo(hh)
                else:
                    # attn(h0) interleaves the rest of phase B, then h1's
                    # units; w(h2) is fetched as a filler so its buffer
                    # rotation follows all w(h0) readers in emission order
                    h1_units = qkv_closures(1, cur_pool)
                    mid = [] if H_ <= 2 else [lambda: fetch_w(2)]
                    attn_head(0, rest + mid + h1_units, need=needs)
                    for h in range(2, H_):         # slots 2..H-1
                        if h + 1 < H_:
                            fetch_w(h + 1)
                        if h == H_ - 1:
                            for hh in range(H_):
                                fetch_wo(hh)
                        units = qkv_closures(h, cur_pool)
                        if h == H_ - 1 and len(units) > 2:
                            reserved = units[-2:]  # fillers for attn(h3) g0/g1
                            units = units[:-2]
                        attn_head(h - 1, units)
                    if H_ == 2:
                        for hh in range(H_):
                            fetch_wo(hh)

            with tc.tile_pool(name="opsps", bufs=2, space="PSUM") as opsps:
                # alternate stage-copy engines: all-on-one-queue delays the
                # latency-critical exp (Act) / norm (DVE) chains of attn(h3)
                eng_rr = [nc.scalar.copy, nc.vector.tensor_copy]

                def oproj_unit(sb):
                    stage = wtile([128, Dm_], bf16, "ostage", 2)
                    tail = sb >= SBK - 2
                    for db in range(NDB):
                        ps = opsps.tile([128, 512], f32, tag="ops", name="ops")
                        for hh in range(H_):
                            nc.tensor.matmul(ps[:],
                                             attnT[hh][:, sb * 128:(sb + 1) * 128],
                                             wot[hh][:, db * 512:(db + 1) * 512],
                                             start=(hh == 0), stop=(hh == H_ - 1))
                        copy = eng_rr[(sb * NDB + db) % len(eng_rr)]
                        copy(stage[:, db * 512:(db + 1) * 512], ps[:])
                        if tail:    # drain the last rows in db chunks on both
                            eng = nc.sync if db % 2 == 0 else nc.scalar
                            eng.dma_start(
                                out_d[sb * 128:(sb + 1) * 128, db * 512:(db + 1) * 512],
                                stage[:, db * 512:(db + 1) * 512])
                    if not tail:
                        eng = nc.sync if sb % 2 == 0 else nc.scalar
                        eng.dma_start(out_d[sb * 128:(sb + 1) * 128, :], stage[:])

                # last head's attention: two reserved qkv units fill the first
                # groups (redirected to the st psum pool now that qkvps is
                # closed), then o_proj units gated per finished group; the
                # last group norms/transposes per chunk so its o_proj units
                # unblock before the group ends
                cur_pool["pool"], cur_pool["tag"] = stps, "st"
                nres = len(reserved)
                filler = reserved + [lambda sb=sb: oproj_unit(sb) for sb in range(SBK)]
                attn_head(H_ - 1, filler, gate=lambda g: nres + 4 * g,
                          eager_last=True)

    nc.compile()
    return nc


# --------------------------------------------------------------------------
# host sharding + entry point
# --------------------------------------------------------------------------

def _prep_core_inputs(x, Wq, Wk, Wv, Wo, fp32r=None):
    """Return list of 8 per-core input dicts (bf16, pre-tiled [128, ...])."""
    import ml_dtypes
    bf = ml_dtypes.bfloat16
    perm = _deinterleave_idx()
    DT = D // 128
    in_maps = []
    for c in range(N_CORES):
        b, g = c // GROUPS, c % GROUPS
        heads = range(g * H_PER_CORE, (g + 1) * H_PER_CORE)
        qk_rows = np.concatenate([h * HD + perm for h in heads])
        v_rows = np.concatenate([np.arange(h * HD, (h + 1) * HD) for h in heads])
        wq_t = Wq[qk_rows, :].T.astype(np.float32)   # [D, E]
        wk_t = Wk[qk_rows, :].T.astype(np.float32)
        wv_t = Wv[v_rows, :].T.astype(np.float32)
        wo_t = Wo[:, v_rows].T.astype(np.float32)    # [E, D]

        # x tiles, d-major: xP[p, d*S+s] = x[b][s, d*128+p]
        xb = np.ascontiguousarray(x[b]).astype(bf)               # [S, D]
        xP = xb.T.reshape(DT, 128, S).transpose(1, 0, 2).reshape(128, DT * S)

        def tile_w(w):
            """w [D, C] -> [128, (C/128)*DT*128]: per 128-col block, DT
            d-tiles of [128, 128] laid out d-major."""
            C = w.shape[1]
            nb = C // 128
            out = np.empty((128, nb * DT * 128), dtype=bf)
            wb = w.astype(bf)
            for bi in range(nb):
                t3 = wb[:, bi * 128:(bi + 1) * 128].reshape(DT, 128, 128)
                out[:, bi * DT * 128:(bi + 1) * DT * 128] = (
                    t3.transpose(1, 0, 2).reshape(128, DT * 128))
            return out

        # wqk blocks in (h, p) order: block 2h = q head h, block 2h+1 = k
        wqk = np.empty((D, 2 * E_PER_CORE), dtype=np.float32)
        for h in range(H_PER_CORE):
            wqk[:, (2 * h) * 128:(2 * h + 1) * 128] = wq_t[:, h * 128:(h + 1) * 128]
            wqk[:, (2 * h + 1) * 128:(2 * h + 2) * 128] = wk_t[:, h * 128:(h + 1) * 128]

        woP = wo_t.reshape(H_PER_CORE, 128, D).transpose(1, 0, 2).reshape(
            128, H_PER_CORE * D).astype(bf)

        in_maps.append({
            "xP": np.ascontiguousarray(xP),
            "wqkP": np.ascontiguousarray(tile_w(wqk)),
            "wvP": np.ascontiguousarray(tile_w(wv_t)),
            "woP": np.ascontiguousarray(woP),
        })
    return in_maps


def _is_causal_mask(attn_mask):
    if attn_mask is None:
        return True
    m = np.asarray(attn_mask)
    if m.shape != (1, 1, S, S):
        return False
    m2 = m[0, 0]
    tril = np.tril(np.ones((S, S), dtype=bool))
    return bool(np.all(m2[tril] == 0.0) and np.all(m2[~tril] <= -1.0e30))


def _get_program(mmdt="bf16"):
    key = ("full", "bf16")
    if key not in _CACHE:
        _CACHE[key] = build(S, D, H_PER_CORE)
    return _CACHE[key]


def run_on_hw(in_maps, mmdt="bf16", trace=False, **kwargs):
    """Run the SPMD program on the 8 NeuronCores; returns BassKernelResults."""
    from concourse.bass_utils import run_bass_kernel_spmd
    nc = _get_program(mmdt)
    return run_bass_kernel_spmd(nc, in_maps, core_ids=list(range(N_CORES)),
                                trace=trace, **kwargs)


def kernel(x, Wq, Wk, Wv, Wo, attn_mask=None, **_ignored):
    x = np.asarray(x, dtype=np.float32)
    Wq = np.asarray(Wq, dtype=np.float32)
    Wk = np.asarray(Wk, dtype=np.float32)
    Wv = np.asarray(Wv, dtype=np.float32)
    Wo = np.asarray(Wo, dtype=np.float32)

    if not _is_causal_mask(attn_mask):
        return _np_reference(x, Wq, Wk, Wv, Wo,
                             np.asarray(attn_mask, dtype=np.float32)).astype(np.float32)

    in_maps = _prep_core_inputs(x, Wq, Wk, Wv, Wo)
    res = run_on_hw(in_maps, trace=False)

    out = np.zeros((B, S, D), dtype=np.float32)
    for c in range(N_CORES):
        out[c // GROUPS] += res.results[c]["out"].astype(np.float32)
    return out
